# revision 27
# baseline (speedup 1.0000x reference)
"""Bahdanau additive attention on 8 TRN2 NeuronCores (Bass/Tile).

Reference computation (B=4, T=512, S=512, D=256, IN=512):
    wq[b,t,d]   = sum_i x[b,t,i]   * Wq[d,i]
    uh[b,s,d]   = sum_m mems[b,s,m]* Wc[d,m] + bc[d]
    align[b,t,s]= sum_d v[d] * tanh(wq[b,t,d] + uh[b,s,d])     (masked s>=L_b -> -inf)
    av          = softmax_s(align)
    c[b,t,m]    = sum_s av[b,t,s] * mems[b,s,m]
    attn[b,t,o] = sum_k [c|x][b,t,k] * Wout[o,k] + bout[o]
    returns (attn, av)

Sharding: 16 (batch, t-tile-of-128) blocks, 2 per core (pure data parallel,
no collectives).  Per (t, d-half): DVE broadcast-add z = uh + wq[t] (bf16,
4x mode), ACT tanh batched over many slices (one big ACTIVATE), PE reduces
over the d-partition dim with a 32-column one-hot v weight into the PSUM
row for t.  Mask is a rank-1 additive matmul on the align PSUM (0 / -30);
softmax sum is fused into the Exp via accum_out; bout is a rank-1 matmul;
the output projection is DMA'd straight from PSUM.  All matmul inputs
bf16, accumulation fp32.  DRAM inputs are pre-laid host-side in the exact
SBUF [128, ...] layout so every DMA is contiguous.
"""
import numpy as np
import ml_dtypes
from contextlib import ExitStack

import concourse.bass as bass
import concourse.bacc as bacc
import concourse.mybir as mybir
import concourse.tile as tile
from concourse.bass_utils import run_bass_kernel_spmd

F32 = mybir.dt.float32
BF16 = mybir.dt.bfloat16
TANH = mybir.ActivationFunctionType.Tanh
EXP = mybir.ActivationFunctionType.Exp
BF = ml_dtypes.bfloat16

B, T, S, D, IN = 4, 512, 512, 256, 512
NC = 8           # cores
NJ = 2           # t-tiles per core
TT = 128         # t rows per tile

# ACT batch sizes (t's per ACTIVATE).  Small groups at the edges: the first
# tanh starts sooner (less z to build) and after the last tanh the PE's
# v-matmul chase is short.
GROUPS_HEAD = [2, 2, 4, 8] + [12] * 9 + [4]
GROUPS_TAIL = [4] + [12] * 9 + [8, 4, 2, 2]
assert sum(GROUPS_HEAD) == TT and sum(GROUPS_TAIL) == TT
GMAX = max(GROUPS_HEAD)

_BUILT = None
LAST_RESULT = None


def _build():
    nc = bacc.Bacc("TRN2", target_bir_lowering=False, debug=False,
                   enable_asserts=False, num_devices=NC)

    # all inputs pre-laid in SBUF layout [128 partitions, free...]
    xT_d = nc.dram_tensor("xT", [NJ, 4, 128, TT], BF16, kind="ExternalInput")
    memsT_d = nc.dram_tensor("memsT", [NJ, 2, 128, S], BF16, kind="ExternalInput")
    memsL_d = nc.dram_tensor("memsL", [NJ, 128, 4, D], BF16, kind="ExternalInput")
    maskadd_d = nc.dram_tensor("maskadd", [NJ, 1, S], BF16, kind="ExternalInput")
    ones_d = nc.dram_tensor("ones1", [1, 128], BF16, kind="ExternalInput")
    boutw_d = nc.dram_tensor("boutw", [1, IN], BF16, kind="ExternalInput")
    WqT_d = nc.dram_tensor("WqT", [4, 128, D], BF16, kind="ExternalInput")
    WcT_d = nc.dram_tensor("WcT", [2, 128, D], BF16, kind="ExternalInput")
    vcols_d = nc.dram_tensor("vcols", [128, 2, 32, 32], BF16, kind="ExternalInput")
    WoCT_d = nc.dram_tensor("WoCT", [128, 2, IN], BF16, kind="ExternalInput")
    WoXT_d = nc.dram_tensor("WoXT", [128, 4, IN], BF16, kind="ExternalInput")
    ident_d = nc.dram_tensor("ident", [128, 128], BF16, kind="ExternalInput")
    bc_d = nc.dram_tensor("bc2", [128, 2], F32, kind="ExternalInput")

    attn_d = nc.dram_tensor("attn_outT", [NJ, 128, 4, TT], F32, kind="ExternalOutput")
    align_d = nc.dram_tensor("align_out", [NJ, 128, S], F32, kind="ExternalOutput")

    with tile.TileContext(nc) as tc, ExitStack() as ctx:
        const = ctx.enter_context(tc.tile_pool(name="const", bufs=1))
        pin = ctx.enter_context(tc.tile_pool(name="pin", bufs=NJ))
        pmid = ctx.enter_context(tc.tile_pool(name="pmid", bufs=NJ))
        pz = ctx.enter_context(tc.tile_pool(name="pz", bufs=3))
        pth = ctx.enter_context(tc.tile_pool(name="pth", bufs=2))
        pep = ctx.enter_context(tc.tile_pool(name="pep", bufs=NJ))
        psA = ctx.enter_context(tc.tile_pool(name="psA", bufs=NJ, space="PSUM"))
        psU = ctx.enter_context(tc.tile_pool(name="psU", bufs=2, space="PSUM"))
        psWC = ctx.enter_context(tc.tile_pool(name="psWC", bufs=1, space="PSUM"))
        psT = ctx.enter_context(tc.tile_pool(name="psT", bufs=2, space="PSUM"))
        psO = ctx.enter_context(tc.tile_pool(name="psO", bufs=1, space="PSUM"))

        def load(pool, shape, dt, src, tag, engine=None):
            t = pool.tile(shape, dt, tag=tag)
            (engine or nc.sync).dma_start(t[...], src)
            return t

        # startup-critical loads (wq path: xT0+WqT, uh path: memsT0+WcT)
        # balanced across the two DMA queues, chunked so each matmul starts
        # as soon as its chunk lands
        bc2 = load(const, [128, 2], F32, bc_d.ap(), "bc2")
        xT0c, wqTc, memsT0c, wcTc = [None] * 4, [None] * 4, [None] * 2, [None] * 2
        wcTc[0] = load(const, [128, D], BF16, WcT_d.ap()[0], "wcTc0", nc.gpsimd)
        memsT0c[0] = load(pin, [128, S], BF16, memsT_d.ap()[0][0], "mT0c0",
                          nc.gpsimd)
        for ic in range(2):
            xT0c[ic] = load(pin, [128, TT], BF16, xT_d.ap()[0][ic], f"xT0c{ic}")
            wqTc[ic] = load(const, [128, D], BF16, WqT_d.ap()[ic], f"wqTc{ic}")
        for ic in range(2, 4):
            xT0c[ic] = load(pin, [128, TT], BF16, xT_d.ap()[0][ic], f"xT0c{ic}",
                            nc.gpsimd)
            wqTc[ic] = load(const, [128, D], BF16, WqT_d.ap()[ic], f"wqTc{ic}",
                            nc.gpsimd)
        memsT0c[1] = load(pin, [128, S], BF16, memsT_d.ap()[0][1], "mT0c1")
        wcTc[1] = load(const, [128, D], BF16, WcT_d.ap()[1], "wcTc1")
        vcols = load(const, [128, 2, 32, 32], BF16, vcols_d.ap(), "vcols")
        maskadds = [load(pin, [1, S], BF16, maskadd_d.ap()[j], "maskadd")
                    for j in range(NJ)]
        ones1 = load(const, [1, 128], BF16, ones_d.ap(), "ones1")
        boutw = load(const, [1, IN], BF16, boutw_d.ap(), "boutw")

        xT1c = [load(pin, [128, TT], BF16, xT_d.ap()[1][ic], f"xT1c{ic}",
                     nc.gpsimd) for ic in range(4)]
        memsT1c = [load(pin, [128, S], BF16, memsT_d.ap()[1][mc], f"mT1c{mc}",
                        nc.gpsimd) for mc in range(2)]
        woCT = load(const, [128, 2, IN], BF16, WoCT_d.ap(), "woCT", nc.gpsimd)
        woXT = load(const, [128, 4, IN], BF16, WoXT_d.ap(), "woXT", nc.gpsimd)
        ident = load(const, [128, 128], BF16, ident_d.ap(), "ident", nc.gpsimd)
        memsLs = [load(pin, [128, 4, D], BF16, memsL_d.ap()[j], "memsL",
                       nc.gpsimd) for j in range(NJ)]
        xTc = [xT0c, xT1c]
        memsTc = [memsT0c, memsT1c]

        def phase1(j):
            """wq [d_half, h, t] and uh [d_half, h, s] for tile j."""
            wq_ps = psWC.tile([128, 2 * TT], F32, tag="wqc_ps")
            for h in range(2):
                for ic in range(4):
                    nc.tensor.matmul(wq_ps[:, h * TT:(h + 1) * TT],
                                     wqTc[ic][:, h * 128:(h + 1) * 128],
                                     xTc[j][ic][...],
                                     start=(ic == 0), stop=(ic == 3))
            wq_sb = pmid.tile([128, 2, TT], F32, tag="wq_sb")
            nc.vector.tensor_copy(wq_sb[...], wq_ps[...])

            uh_sb = pmid.tile([128, 2, S], BF16, tag="uh_sb")
            for h in range(2):
                uh_ps = psU.tile([128, S], F32, tag="uh_ps")
                for mc in range(2):
                    nc.tensor.matmul(uh_ps[...],
                                     wcTc[mc][:, h * 128:(h + 1) * 128],
                                     memsTc[j][mc][...],
                                     start=(mc == 0), stop=(mc == 1))
                nc.vector.tensor_scalar_add(uh_sb[:, h, :], uh_ps[...],
                                            bc2[:, h:h + 1])
            return wq_sb, uh_sb

        p1 = [phase1(0), None]
        groups_of = [GROUPS_HEAD, GROUPS_TAIL]
        toff = [np.cumsum([0] + g).tolist() for g in groups_of]
        align_pss = [None, None]

        def emit_group(j, gi):
            wq_sb, uh_sb = p1[j]
            gsz = groups_of[j][gi]
            t0 = toff[j][gi]
            if align_pss[j] is None:
                align_pss[j] = psA.tile([128, S], F32, tag="align_ps",
                                        name=f"align_ps{j}")
            align_ps = align_pss[j]
            z = pz.tile([128, gsz * 2 * S], BF16, tag="z",
                        padded_shape=[128, GMAX * 2 * S])
            for jj in range(gsz):
                t = t0 + jj
                for h in range(2):
                    nc.vector.tensor_scalar_add(
                        z[:, (2 * jj + h) * S:(2 * jj + h + 1) * S],
                        uh_sb[:, h, :], wq_sb[:, h, t:t + 1])
            th = pth.tile([128, gsz * 2 * S], BF16, tag="th",
                          padded_shape=[128, GMAX * 2 * S])
            nc.scalar.activation(th[...], z[...], TANH)
            for jj in range(gsz):
                t = t0 + jj
                k, c = t // 32, t % 32
                for h in range(2):
                    nc.tensor.matmul(
                        align_ps[32 * k:32 * (k + 1), :],
                        vcols[:, h, c, :],
                        th[:, (2 * jj + h) * S:(2 * jj + h + 1) * S],
                        start=(c == 0 and h == 0),
                        stop=(c == 31 and h == 1),
                        tile_position=(0, 32 * k))

        def emit_epilogue(j):
            memsL = memsLs[j]
            align_ps = align_pss[j]

            # additive mask into PSUM: align += ones^T @ maskadd (0 / -30)
            nc.tensor.matmul(align_ps[...], ones1[...], maskadds[j][...],
                             start=False, stop=True, skip_group_check=True)

            # softmax (no max-sub: |align| <= ~10); sum fused into the exp
            av_e = pep.tile([128, S], F32, tag="av_e")
            ssum = pep.tile([128, 1], F32, tag="ssum")
            nc.scalar.activation(av_e[...], align_ps[...], EXP,
                                 accum_out=ssum[...])

            # x-part of the output projection does not depend on softmax;
            # one accumulation group for the whole bank (start=True clears
            # has_written bank-wide, so only the very first matmul starts)
            at_ps = psO.tile([128, 4 * TT], F32, tag="at_ps")
            for oc in range(4):
                for ic in range(4):
                    nc.tensor.matmul(at_ps[:, oc * TT:(oc + 1) * TT],
                                     woXT[:, ic, oc * 128:(oc + 1) * 128],
                                     xTc[j][ic][...],
                                     start=(oc == 0 and ic == 0), stop=False)

            rcp = pep.tile([128, 1], F32, tag="rcp")
            nc.vector.reciprocal(rcp[...], ssum[...])
            # bf16 copy first: it gates the transpose -> c chain
            av_bf = pep.tile([128, S], BF16, tag="av_bf")
            nc.vector.tensor_scalar_mul(av_bf[...], av_e[...], rcp[...])

            # transpose av -> [s, t]; one tile per s-block so the c matmul
            # can start as soon as its block is ready
            avTs = []
            for sb in range(4):
                tp = psT.tile([128, 128], BF16, tag="tp")
                nc.tensor.transpose(tp[...], av_bf[:, sb * 128:(sb + 1) * 128],
                                    ident[...])
                avT = pep.tile([128, TT], BF16, tag=f"avT{sb}")
                nc.vector.tensor_copy(avT[...], tp[...])
                avTs.append(avT)

            # align_vectors output (overlaps the c matmuls)
            av = pep.tile([128, S], F32, tag="av")
            nc.vector.tensor_scalar_mul(av[...], av_e[...], rcp[...])
            nc.sync.dma_start(align_d.ap()[j], av[...])

            # c[t,m] laid out [m_half(128), mh, t]
            c_ps = psWC.tile([128, 2 * TT], F32, tag="wqc_ps")
            for mh in range(2):
                for sb in range(4):
                    nc.tensor.matmul(c_ps[:, mh * TT:(mh + 1) * TT],
                                     memsL[:, sb, mh * 128:(mh + 1) * 128],
                                     avTs[sb][...],
                                     start=(sb == 0), stop=(sb == 3))
            c_bf = pep.tile([128, 2, TT], BF16, tag="c_bf")
            nc.vector.tensor_copy(c_bf[...], c_ps[...])

            # c-part + bout accumulate onto the x-part; per-oc chains so
            # eviction and output DMA pipeline with the remaining matmuls
            for oc in range(4):
                for mh in range(2):
                    nc.tensor.matmul(at_ps[:, oc * TT:(oc + 1) * TT],
                                     woCT[:, mh, oc * 128:(oc + 1) * 128],
                                     c_bf[:, mh, :],
                                     start=False, stop=False)
                nc.tensor.matmul(at_ps[:, oc * TT:(oc + 1) * TT],
                                 boutw[:, oc * 128:(oc + 1) * 128],
                                 ones1[...],
                                 start=False, stop=(oc == 3),
                                 skip_group_check=True)
                attn_sb = pep.tile([128, TT], F32, tag=f"attn_sb{oc}")
                nc.vector.tensor_copy(attn_sb[...],
                                      at_ps[:, oc * TT:(oc + 1) * TT])
                nc.sync.dma_start(attn_d.ap()[j][:, oc, :], attn_sb[...])

        # schedule: tile-0 groups (tile-1 prologue injected at group 6);
        # tile-1's first groups are emitted before tile-0's epilogue so the
        # DVE keeps feeding the ACT across the tile transition
        for gi in range(len(GROUPS_HEAD)):
            if gi == 6:
                p1[1] = phase1(1)
            emit_group(0, gi)
        emit_group(1, 0)
        emit_group(1, 1)
        emit_epilogue(0)
        for gi in range(2, len(GROUPS_TAIL)):
            emit_group(1, gi)
        emit_epilogue(1)

    nc.compile()
    return nc


def _to_chunks(a, nch):
    """[nch*128, F] -> [nch, 128, F] (partition-chunked SBUF layout)."""
    return np.ascontiguousarray(a.reshape(nch, 128, a.shape[-1]))


def _to_pcf(a, nch):
    """[nch*128, F] -> [128, nch, F] (single-tile chunked free layout)."""
    return np.ascontiguousarray(a.reshape(nch, 128, a.shape[-1]).transpose(1, 0, 2))


def _prep_inputs(inputs, mems, mem_masks, Wq, Wc, bc, v, Wout, bout):
    x = np.ascontiguousarray(np.asarray(inputs, dtype=np.float32))
    mems = np.ascontiguousarray(np.asarray(mems, dtype=np.float32))
    L = np.asarray(mem_masks).astype(np.int64)
    Wq = np.asarray(Wq, dtype=np.float32)
    Wc = np.asarray(Wc, dtype=np.float32)
    bc = np.asarray(bc, dtype=np.float32)
    v = np.asarray(v, dtype=np.float32)
    Wout = np.asarray(Wout, dtype=np.float32)
    bout = np.asarray(bout, dtype=np.float32)

    WqT = _to_chunks(np.ascontiguousarray(Wq.T), 4).astype(BF)      # [4,128,D]
    WcT = _to_chunks(np.ascontiguousarray(Wc.T), 2).astype(BF)      # [2,128,D]
    WoCT = _to_pcf(np.ascontiguousarray(Wout[:, :D].T), 2).astype(BF)
    WoXT = _to_pcf(np.ascontiguousarray(Wout[:, D:].T), 4).astype(BF)
    ident = np.eye(128, dtype=np.float32).astype(BF)
    bc2 = np.ascontiguousarray(bc.reshape(2, 128).T).astype(np.float32)
    vcols = np.zeros((128, 2, 32, 32), np.float32)
    for h in range(2):
        for c in range(32):
            vcols[:, h, c, c] = v[h * 128:(h + 1) * 128]
    vcols = vcols.astype(BF)

    shared = dict(WqT=WqT, WcT=WcT, vcols=vcols, WoCT=WoCT, WoXT=WoXT,
                  ident=ident, bc2=bc2,
                  boutw=bout.reshape(1, IN).astype(BF),
                  ones1=np.ones((1, 128), np.float32).astype(BF))

    tiles = [(b, tt) for b in range(B) for tt in range(T // TT)]
    in_maps = []
    for core in range(NC):
        xT = np.zeros((NJ, 4, 128, TT), np.float32)
        memsT = np.zeros((NJ, 2, 128, S), np.float32)
        memsL = np.zeros((NJ, 128, 4, D), np.float32)
        maskadd = np.zeros((NJ, 1, S), np.float32)
        for j in range(NJ):
            b, tt = tiles[core * NJ + j]
            xT[j] = _to_chunks(
                np.ascontiguousarray(x[b, tt * TT:(tt + 1) * TT, :].T), 4)
            memsT[j] = _to_chunks(np.ascontiguousarray(mems[b].T), 2)
            memsL[j] = _to_pcf(mems[b], 4)
            maskadd[j, 0, :] = np.where(np.arange(S) < int(L[b]), 0.0, -30.0)
        m = dict(shared)
        m["xT"] = np.ascontiguousarray(xT).astype(BF)
        m["memsT"] = np.ascontiguousarray(memsT).astype(BF)
        m["memsL"] = np.ascontiguousarray(memsL).astype(BF)
        m["maskadd"] = np.ascontiguousarray(maskadd).astype(BF)
        in_maps.append(m)
    return in_maps, tiles


def kernel(**inputs):
    global _BUILT, LAST_RESULT
    in_maps, tiles = _prep_inputs(**inputs)
    if _BUILT is None:
        _BUILT = _build()
    res = run_bass_kernel_spmd(_BUILT, in_maps, core_ids=list(range(NC)))
    LAST_RESULT = res

    attn_h = np.zeros((B, T, IN), np.float32)
    align_v = np.zeros((B, T, S), np.float32)
    for core in range(NC):
        for j in range(NJ):
            b, tt = tiles[core * NJ + j]
            at = res.results[core]["attn_outT"][j]        # [128(p), 4(oc), 128(t)]
            attn_h[b, tt * TT:(tt + 1) * TT, :] = \
                np.transpose(at, (2, 1, 0)).reshape(TT, IN)
            align_v[b, tt * TT:(tt + 1) * TT, :] = res.results[core]["align_out"][j]
    return attn_h, align_v


# revision 28
# speedup vs baseline: 1.0178x; 1.0178x over previous
"""Bahdanau additive attention on 8 TRN2 NeuronCores (Bass/Tile).

Reference computation (B=4, T=512, S=512, D=256, IN=512):
    wq[b,t,d]   = sum_i x[b,t,i]   * Wq[d,i]
    uh[b,s,d]   = sum_m mems[b,s,m]* Wc[d,m] + bc[d]
    align[b,t,s]= sum_d v[d] * tanh(wq[b,t,d] + uh[b,s,d])     (masked s>=L_b -> -inf)
    av          = softmax_s(align)
    c[b,t,m]    = sum_s av[b,t,s] * mems[b,s,m]
    attn[b,t,o] = sum_k [c|x][b,t,k] * Wout[o,k] + bout[o]
    returns (attn, av)

Sharding: 16 (batch, t-tile-of-128) blocks, 2 per core (pure data parallel,
no collectives).  Per (t, d-half): DVE broadcast-add z = uh + wq[t] (bf16,
4x mode), ACT tanh batched over many slices (one big ACTIVATE), PE reduces
over the d-partition dim with a 32-column one-hot v weight into the PSUM
row for t.  Mask is a rank-1 additive matmul on the align PSUM (0 / -30);
softmax sum is fused into the Exp via accum_out; bout is a rank-1 matmul;
the output projection is DMA'd straight from PSUM.  All matmul inputs
bf16, accumulation fp32.  DRAM inputs are pre-laid host-side in the exact
SBUF [128, ...] layout so every DMA is contiguous.
"""
import numpy as np
import ml_dtypes
from contextlib import ExitStack

import concourse.bass as bass
import concourse.bacc as bacc
import concourse.mybir as mybir
import concourse.tile as tile
from concourse.bass_utils import run_bass_kernel_spmd

F32 = mybir.dt.float32
BF16 = mybir.dt.bfloat16
TANH = mybir.ActivationFunctionType.Tanh
EXP = mybir.ActivationFunctionType.Exp
BF = ml_dtypes.bfloat16

B, T, S, D, IN = 4, 512, 512, 256, 512
NC = 8           # cores
NJ = 2           # t-tiles per core
TT = 128         # t rows per tile

# ACT batch sizes (t's per ACTIVATE).  Small groups at the edges: the first
# tanh starts sooner (less z to build) and after the last tanh the PE's
# v-matmul chase is short.
GROUPS_HEAD = [2, 2, 4] + [8] * 15
GROUPS_TAIL = [8] * 15 + [4, 2, 2]
assert sum(GROUPS_HEAD) == TT and sum(GROUPS_TAIL) == TT
GMAX = max(GROUPS_HEAD)

_BUILT = None
LAST_RESULT = None


def _build():
    nc = bacc.Bacc("TRN2", target_bir_lowering=False, debug=False,
                   enable_asserts=False, num_devices=NC)

    # all inputs pre-laid in SBUF layout [128 partitions, free...]
    xT_d = nc.dram_tensor("xT", [NJ, 4, 128, TT], BF16, kind="ExternalInput")
    memsT_d = nc.dram_tensor("memsT", [NJ, 2, 128, S], BF16, kind="ExternalInput")
    memsL_d = nc.dram_tensor("memsL", [NJ, 128, 4, D], BF16, kind="ExternalInput")
    maskadd_d = nc.dram_tensor("maskadd", [NJ, 1, S], BF16, kind="ExternalInput")
    ones_d = nc.dram_tensor("ones1", [1, 128], BF16, kind="ExternalInput")
    boutw_d = nc.dram_tensor("boutw", [1, IN], BF16, kind="ExternalInput")
    WqT_d = nc.dram_tensor("WqT", [4, 128, D], BF16, kind="ExternalInput")
    WcT_d = nc.dram_tensor("WcT", [2, 128, D], BF16, kind="ExternalInput")
    vcols_d = nc.dram_tensor("vcols", [128, 2, 32, 32], BF16, kind="ExternalInput")
    WoCT_d = nc.dram_tensor("WoCT", [128, 2, IN], BF16, kind="ExternalInput")
    WoXT_d = nc.dram_tensor("WoXT", [128, 4, IN], BF16, kind="ExternalInput")
    ident_d = nc.dram_tensor("ident", [128, 128], BF16, kind="ExternalInput")
    bc_d = nc.dram_tensor("bc2", [128, 2], F32, kind="ExternalInput")

    attn_d = nc.dram_tensor("attn_outT", [NJ, 128, 4, TT], F32, kind="ExternalOutput")
    align_d = nc.dram_tensor("align_out", [NJ, 128, S], F32, kind="ExternalOutput")

    with tile.TileContext(nc) as tc, ExitStack() as ctx:
        const = ctx.enter_context(tc.tile_pool(name="const", bufs=1))
        pin = ctx.enter_context(tc.tile_pool(name="pin", bufs=NJ))
        pmid = ctx.enter_context(tc.tile_pool(name="pmid", bufs=NJ))
        pz = ctx.enter_context(tc.tile_pool(name="pz", bufs=4))
        pth = ctx.enter_context(tc.tile_pool(name="pth", bufs=4))
        pep = ctx.enter_context(tc.tile_pool(name="pep", bufs=NJ))
        psA = ctx.enter_context(tc.tile_pool(name="psA", bufs=NJ, space="PSUM"))
        psU = ctx.enter_context(tc.tile_pool(name="psU", bufs=2, space="PSUM"))
        psWC = ctx.enter_context(tc.tile_pool(name="psWC", bufs=1, space="PSUM"))
        psT = ctx.enter_context(tc.tile_pool(name="psT", bufs=2, space="PSUM"))
        psO = ctx.enter_context(tc.tile_pool(name="psO", bufs=1, space="PSUM"))

        def load(pool, shape, dt, src, tag, engine=None):
            t = pool.tile(shape, dt, tag=tag)
            (engine or nc.sync).dma_start(t[...], src)
            return t

        # startup-critical loads (wq path: xT0+WqT, uh path: memsT0+WcT)
        # balanced across the two DMA queues, chunked so each matmul starts
        # as soon as its chunk lands
        bc2 = load(const, [128, 2], F32, bc_d.ap(), "bc2")
        xT0c, wqTc, memsT0c, wcTc = [None] * 4, [None] * 4, [None] * 2, [None] * 2
        wcTc[0] = load(const, [128, D], BF16, WcT_d.ap()[0], "wcTc0", nc.gpsimd)
        memsT0c[0] = load(pin, [128, S], BF16, memsT_d.ap()[0][0], "mT0c0",
                          nc.gpsimd)
        for ic in range(2):
            xT0c[ic] = load(pin, [128, TT], BF16, xT_d.ap()[0][ic], f"xT0c{ic}")
            wqTc[ic] = load(const, [128, D], BF16, WqT_d.ap()[ic], f"wqTc{ic}")
        for ic in range(2, 4):
            xT0c[ic] = load(pin, [128, TT], BF16, xT_d.ap()[0][ic], f"xT0c{ic}",
                            nc.gpsimd)
            wqTc[ic] = load(const, [128, D], BF16, WqT_d.ap()[ic], f"wqTc{ic}",
                            nc.gpsimd)
        memsT0c[1] = load(pin, [128, S], BF16, memsT_d.ap()[0][1], "mT0c1")
        wcTc[1] = load(const, [128, D], BF16, WcT_d.ap()[1], "wcTc1")
        vcols = load(const, [128, 2, 32, 32], BF16, vcols_d.ap(), "vcols")
        maskadds = [load(pin, [1, S], BF16, maskadd_d.ap()[j], "maskadd")
                    for j in range(NJ)]
        ones1 = load(const, [1, 128], BF16, ones_d.ap(), "ones1")
        boutw = load(const, [1, IN], BF16, boutw_d.ap(), "boutw")

        xT1c = [load(pin, [128, TT], BF16, xT_d.ap()[1][ic], f"xT1c{ic}",
                     nc.gpsimd) for ic in range(4)]
        memsT1c = [load(pin, [128, S], BF16, memsT_d.ap()[1][mc], f"mT1c{mc}",
                        nc.gpsimd) for mc in range(2)]
        woCT = load(const, [128, 2, IN], BF16, WoCT_d.ap(), "woCT", nc.gpsimd)
        woXT = load(const, [128, 4, IN], BF16, WoXT_d.ap(), "woXT", nc.gpsimd)
        ident = load(const, [128, 128], BF16, ident_d.ap(), "ident", nc.gpsimd)
        memsLs = [load(pin, [128, 4, D], BF16, memsL_d.ap()[j], "memsL",
                       nc.gpsimd) for j in range(NJ)]
        xTc = [xT0c, xT1c]
        memsTc = [memsT0c, memsT1c]

        def phase1(j):
            """wq [d_half, h, t] and uh [d_half, h, s] for tile j."""
            wq_ps = psWC.tile([128, 2 * TT], F32, tag="wqc_ps")
            for h in range(2):
                for ic in range(4):
                    nc.tensor.matmul(wq_ps[:, h * TT:(h + 1) * TT],
                                     wqTc[ic][:, h * 128:(h + 1) * 128],
                                     xTc[j][ic][...],
                                     start=(ic == 0), stop=(ic == 3))
            wq_sb = pmid.tile([128, 2, TT], F32, tag="wq_sb")
            nc.vector.tensor_copy(wq_sb[...], wq_ps[...])

            uh_sb = pmid.tile([128, 2, S], BF16, tag="uh_sb")
            for h in range(2):
                uh_ps = psU.tile([128, S], F32, tag="uh_ps")
                for mc in range(2):
                    nc.tensor.matmul(uh_ps[...],
                                     wcTc[mc][:, h * 128:(h + 1) * 128],
                                     memsTc[j][mc][...],
                                     start=(mc == 0), stop=(mc == 1))
                nc.vector.tensor_scalar_add(uh_sb[:, h, :], uh_ps[...],
                                            bc2[:, h:h + 1])
            return wq_sb, uh_sb

        p1 = [phase1(0), None]
        groups_of = [GROUPS_HEAD, GROUPS_TAIL]
        toff = [np.cumsum([0] + g).tolist() for g in groups_of]
        align_pss = [None, None]

        def emit_group(j, gi):
            wq_sb, uh_sb = p1[j]
            gsz = groups_of[j][gi]
            t0 = toff[j][gi]
            if align_pss[j] is None:
                align_pss[j] = psA.tile([128, S], F32, tag="align_ps",
                                        name=f"align_ps{j}")
            align_ps = align_pss[j]
            z = pz.tile([128, gsz * 2 * S], BF16, tag="z",
                        padded_shape=[128, GMAX * 2 * S])
            for jj in range(gsz):
                t = t0 + jj
                for h in range(2):
                    nc.vector.tensor_scalar_add(
                        z[:, (2 * jj + h) * S:(2 * jj + h + 1) * S],
                        uh_sb[:, h, :], wq_sb[:, h, t:t + 1])
            th = pth.tile([128, gsz * 2 * S], BF16, tag="th",
                          padded_shape=[128, GMAX * 2 * S])
            nc.scalar.activation(th[...], z[...], TANH)
            for jj in range(gsz):
                t = t0 + jj
                k, c = t // 32, t % 32
                for h in range(2):
                    nc.tensor.matmul(
                        align_ps[32 * k:32 * (k + 1), :],
                        vcols[:, h, c, :],
                        th[:, (2 * jj + h) * S:(2 * jj + h + 1) * S],
                        start=(c == 0 and h == 0),
                        stop=(c == 31 and h == 1),
                        tile_position=(0, 32 * k))

        def emit_epilogue(j):
            memsL = memsLs[j]
            align_ps = align_pss[j]

            # additive mask into PSUM: align += ones^T @ maskadd (0 / -30)
            nc.tensor.matmul(align_ps[...], ones1[...], maskadds[j][...],
                             start=False, stop=True, skip_group_check=True)

            # softmax (no max-sub: |align| <= ~10); sum fused into the exp
            av_e = pep.tile([128, S], F32, tag="av_e")
            ssum = pep.tile([128, 1], F32, tag="ssum")
            nc.scalar.activation(av_e[...], align_ps[...], EXP,
                                 accum_out=ssum[...])

            # x-part of the output projection does not depend on softmax;
            # one accumulation group for the whole bank (start=True clears
            # has_written bank-wide, so only the very first matmul starts)
            at_ps = psO.tile([128, 4 * TT], F32, tag="at_ps")
            for oc in range(4):
                for ic in range(4):
                    nc.tensor.matmul(at_ps[:, oc * TT:(oc + 1) * TT],
                                     woXT[:, ic, oc * 128:(oc + 1) * 128],
                                     xTc[j][ic][...],
                                     start=(oc == 0 and ic == 0), stop=False)

            rcp = pep.tile([128, 1], F32, tag="rcp")
            nc.vector.reciprocal(rcp[...], ssum[...])
            # bf16 copy first: it gates the transpose -> c chain
            av_bf = pep.tile([128, S], BF16, tag="av_bf")
            nc.vector.tensor_scalar_mul(av_bf[...], av_e[...], rcp[...])

            # transpose av -> [s, t]; one tile per s-block so the c matmul
            # can start as soon as its block is ready
            avTs = []
            for sb in range(4):
                tp = psT.tile([128, 128], BF16, tag="tp")
                nc.tensor.transpose(tp[...], av_bf[:, sb * 128:(sb + 1) * 128],
                                    ident[...])
                avT = pep.tile([128, TT], BF16, tag=f"avT{sb}")
                nc.vector.tensor_copy(avT[...], tp[...])
                avTs.append(avT)

            # align_vectors output (overlaps the c matmuls)
            av = pep.tile([128, S], F32, tag="av")
            nc.vector.tensor_scalar_mul(av[...], av_e[...], rcp[...])
            nc.sync.dma_start(align_d.ap()[j], av[...])

            # c[t,m] laid out [m_half(128), mh, t]
            c_ps = psWC.tile([128, 2 * TT], F32, tag="wqc_ps")
            for mh in range(2):
                for sb in range(4):
                    nc.tensor.matmul(c_ps[:, mh * TT:(mh + 1) * TT],
                                     memsL[:, sb, mh * 128:(mh + 1) * 128],
                                     avTs[sb][...],
                                     start=(sb == 0), stop=(sb == 3))
            c_bf = pep.tile([128, 2, TT], BF16, tag="c_bf")
            nc.vector.tensor_copy(c_bf[...], c_ps[...])

            # c-part + bout accumulate onto the x-part; per-oc chains so
            # eviction and output DMA pipeline with the remaining matmuls
            for oc in range(4):
                for mh in range(2):
                    nc.tensor.matmul(at_ps[:, oc * TT:(oc + 1) * TT],
                                     woCT[:, mh, oc * 128:(oc + 1) * 128],
                                     c_bf[:, mh, :],
                                     start=False, stop=False)
                nc.tensor.matmul(at_ps[:, oc * TT:(oc + 1) * TT],
                                 boutw[:, oc * 128:(oc + 1) * 128],
                                 ones1[...],
                                 start=False, stop=(oc == 3),
                                 skip_group_check=True)
                attn_sb = pep.tile([128, TT], F32, tag=f"attn_sb{oc}")
                nc.vector.tensor_copy(attn_sb[...],
                                      at_ps[:, oc * TT:(oc + 1) * TT])
                nc.sync.dma_start(attn_d.ap()[j][:, oc, :], attn_sb[...])

        # schedule: tile-0 groups (tile-1 prologue injected at group 6);
        # tile-1's first groups are emitted before tile-0's epilogue so the
        # DVE keeps feeding the ACT across the tile transition
        for gi in range(len(GROUPS_HEAD)):
            if gi == 6:
                p1[1] = phase1(1)
            emit_group(0, gi)
        emit_group(1, 0)
        emit_group(1, 1)
        emit_epilogue(0)
        for gi in range(2, len(GROUPS_TAIL)):
            emit_group(1, gi)
        emit_epilogue(1)

    nc.compile()
    return nc


def _to_chunks(a, nch):
    """[nch*128, F] -> [nch, 128, F] (partition-chunked SBUF layout)."""
    return np.ascontiguousarray(a.reshape(nch, 128, a.shape[-1]))


def _to_pcf(a, nch):
    """[nch*128, F] -> [128, nch, F] (single-tile chunked free layout)."""
    return np.ascontiguousarray(a.reshape(nch, 128, a.shape[-1]).transpose(1, 0, 2))


def _prep_inputs(inputs, mems, mem_masks, Wq, Wc, bc, v, Wout, bout):
    x = np.ascontiguousarray(np.asarray(inputs, dtype=np.float32))
    mems = np.ascontiguousarray(np.asarray(mems, dtype=np.float32))
    L = np.asarray(mem_masks).astype(np.int64)
    Wq = np.asarray(Wq, dtype=np.float32)
    Wc = np.asarray(Wc, dtype=np.float32)
    bc = np.asarray(bc, dtype=np.float32)
    v = np.asarray(v, dtype=np.float32)
    Wout = np.asarray(Wout, dtype=np.float32)
    bout = np.asarray(bout, dtype=np.float32)

    WqT = _to_chunks(np.ascontiguousarray(Wq.T), 4).astype(BF)      # [4,128,D]
    WcT = _to_chunks(np.ascontiguousarray(Wc.T), 2).astype(BF)      # [2,128,D]
    WoCT = _to_pcf(np.ascontiguousarray(Wout[:, :D].T), 2).astype(BF)
    WoXT = _to_pcf(np.ascontiguousarray(Wout[:, D:].T), 4).astype(BF)
    ident = np.eye(128, dtype=np.float32).astype(BF)
    bc2 = np.ascontiguousarray(bc.reshape(2, 128).T).astype(np.float32)
    vcols = np.zeros((128, 2, 32, 32), np.float32)
    for h in range(2):
        for c in range(32):
            vcols[:, h, c, c] = v[h * 128:(h + 1) * 128]
    vcols = vcols.astype(BF)

    shared = dict(WqT=WqT, WcT=WcT, vcols=vcols, WoCT=WoCT, WoXT=WoXT,
                  ident=ident, bc2=bc2,
                  boutw=bout.reshape(1, IN).astype(BF),
                  ones1=np.ones((1, 128), np.float32).astype(BF))

    tiles = [(b, tt) for b in range(B) for tt in range(T // TT)]
    in_maps = []
    for core in range(NC):
        xT = np.zeros((NJ, 4, 128, TT), np.float32)
        memsT = np.zeros((NJ, 2, 128, S), np.float32)
        memsL = np.zeros((NJ, 128, 4, D), np.float32)
        maskadd = np.zeros((NJ, 1, S), np.float32)
        for j in range(NJ):
            b, tt = tiles[core * NJ + j]
            xT[j] = _to_chunks(
                np.ascontiguousarray(x[b, tt * TT:(tt + 1) * TT, :].T), 4)
            memsT[j] = _to_chunks(np.ascontiguousarray(mems[b].T), 2)
            memsL[j] = _to_pcf(mems[b], 4)
            maskadd[j, 0, :] = np.where(np.arange(S) < int(L[b]), 0.0, -30.0)
        m = dict(shared)
        m["xT"] = np.ascontiguousarray(xT).astype(BF)
        m["memsT"] = np.ascontiguousarray(memsT).astype(BF)
        m["memsL"] = np.ascontiguousarray(memsL).astype(BF)
        m["maskadd"] = np.ascontiguousarray(maskadd).astype(BF)
        in_maps.append(m)
    return in_maps, tiles


def kernel(**inputs):
    global _BUILT, LAST_RESULT
    in_maps, tiles = _prep_inputs(**inputs)
    if _BUILT is None:
        _BUILT = _build()
    res = run_bass_kernel_spmd(_BUILT, in_maps, core_ids=list(range(NC)))
    LAST_RESULT = res

    attn_h = np.zeros((B, T, IN), np.float32)
    align_v = np.zeros((B, T, S), np.float32)
    for core in range(NC):
        for j in range(NJ):
            b, tt = tiles[core * NJ + j]
            at = res.results[core]["attn_outT"][j]        # [128(p), 4(oc), 128(t)]
            attn_h[b, tt * TT:(tt + 1) * TT, :] = \
                np.transpose(at, (2, 1, 0)).reshape(TT, IN)
            align_v[b, tt * TT:(tt + 1) * TT, :] = res.results[core]["align_out"][j]
    return attn_h, align_v


# revision 29
# speedup vs baseline: 1.0243x; 1.0064x over previous
"""Bahdanau additive attention on 8 TRN2 NeuronCores (Bass/Tile).

Reference computation (B=4, T=512, S=512, D=256, IN=512):
    wq[b,t,d]   = sum_i x[b,t,i]   * Wq[d,i]
    uh[b,s,d]   = sum_m mems[b,s,m]* Wc[d,m] + bc[d]
    align[b,t,s]= sum_d v[d] * tanh(wq[b,t,d] + uh[b,s,d])     (masked s>=L_b -> -inf)
    av          = softmax_s(align)
    c[b,t,m]    = sum_s av[b,t,s] * mems[b,s,m]
    attn[b,t,o] = sum_k [c|x][b,t,k] * Wout[o,k] + bout[o]
    returns (attn, av)

Sharding: 16 (batch, t-tile-of-128) blocks, 2 per core (pure data parallel,
no collectives).  Per (t, d-half): DVE broadcast-add z = uh + wq[t] (bf16,
4x mode), ACT tanh batched over many slices (one big ACTIVATE), PE reduces
over the d-partition dim with a 32-column one-hot v weight into the PSUM
row for t.  Mask is a rank-1 additive matmul on the align PSUM (0 / -30);
softmax sum is fused into the Exp via accum_out; bout is a rank-1 matmul.
All matmul inputs bf16, accumulation fp32.  DRAM inputs are pre-laid
host-side in the exact SBUF [128, ...] layout so every DMA is contiguous.
"""
import numpy as np
import ml_dtypes
from contextlib import ExitStack

import concourse.bass as bass
import concourse.bacc as bacc
import concourse.mybir as mybir
import concourse.tile as tile
from concourse.bass_utils import run_bass_kernel_spmd

F32 = mybir.dt.float32
BF16 = mybir.dt.bfloat16
TANH = mybir.ActivationFunctionType.Tanh
EXP = mybir.ActivationFunctionType.Exp
BF = ml_dtypes.bfloat16

B, T, S, D, IN = 4, 512, 512, 256, 512
NC = 8           # cores
NJ = 2           # t-tiles per core
TT = 128         # t rows per tile

# ACT batch sizes (t's per ACTIVATE).  Small groups at the edges: the first
# tanh starts sooner (less z to build) and after the last tanh the PE's
# v-matmul chase is short.
GROUPS_HEAD = [2, 2, 4] + [8] * 15
GROUPS_TAIL = [8] * 15 + [4, 2, 2]
assert sum(GROUPS_HEAD) == TT and sum(GROUPS_TAIL) == TT
GMAX = max(GROUPS_HEAD)

_BUILT = None
LAST_RESULT = None


def _build():
    nc = bacc.Bacc("TRN2", target_bir_lowering=False, debug=False,
                   enable_asserts=False, num_devices=NC)

    # all inputs pre-laid in SBUF layout [128 partitions, free...]
    xT_d = nc.dram_tensor("xT", [NJ, 4, 128, TT], BF16, kind="ExternalInput")
    memsT_d = nc.dram_tensor("memsT", [NJ, 2, 128, S], BF16, kind="ExternalInput")
    memsL_d = nc.dram_tensor("memsL", [NJ, 128, 4, D], BF16, kind="ExternalInput")
    maskadd_d = nc.dram_tensor("maskadd", [NJ, 1, S], BF16, kind="ExternalInput")
    ones_d = nc.dram_tensor("ones1", [1, 128], BF16, kind="ExternalInput")
    boutw_d = nc.dram_tensor("boutw", [1, IN], BF16, kind="ExternalInput")
    WqT_d = nc.dram_tensor("WqT", [4, 128, D], BF16, kind="ExternalInput")
    WcT_d = nc.dram_tensor("WcT", [2, 128, D], BF16, kind="ExternalInput")
    vcols_d = nc.dram_tensor("vcols", [128, 2, 32, 32], BF16, kind="ExternalInput")
    WoCT_d = nc.dram_tensor("WoCT", [128, 2, IN], BF16, kind="ExternalInput")
    WoXT_d = nc.dram_tensor("WoXT", [128, 4, IN], BF16, kind="ExternalInput")
    ident_d = nc.dram_tensor("ident", [128, 128], BF16, kind="ExternalInput")
    bc_d = nc.dram_tensor("bc2", [128, 2], F32, kind="ExternalInput")

    attn_d = nc.dram_tensor("attn_outT", [NJ, 128, 4, TT], F32, kind="ExternalOutput")
    align_d = nc.dram_tensor("align_out", [NJ, 128, S], F32, kind="ExternalOutput")

    with tile.TileContext(nc) as tc, ExitStack() as ctx:
        const = ctx.enter_context(tc.tile_pool(name="const", bufs=1))
        pin = ctx.enter_context(tc.tile_pool(name="pin", bufs=NJ))
        pmid = ctx.enter_context(tc.tile_pool(name="pmid", bufs=NJ))
        pz = ctx.enter_context(tc.tile_pool(name="pz", bufs=4))
        pth = ctx.enter_context(tc.tile_pool(name="pth", bufs=4))
        pep = ctx.enter_context(tc.tile_pool(name="pep", bufs=NJ))
        psA = ctx.enter_context(tc.tile_pool(name="psA", bufs=NJ, space="PSUM"))
        psU = ctx.enter_context(tc.tile_pool(name="psU", bufs=2, space="PSUM"))
        psWC = ctx.enter_context(tc.tile_pool(name="psWC", bufs=1, space="PSUM"))
        psT = ctx.enter_context(tc.tile_pool(name="psT", bufs=2, space="PSUM"))
        psO = ctx.enter_context(tc.tile_pool(name="psO", bufs=1, space="PSUM"))

        def load(pool, shape, dt, src, tag, engine=None):
            t = pool.tile(shape, dt, tag=tag)
            (engine or nc.sync).dma_start(t[...], src)
            return t

        # startup-critical loads (wq path: xT0+WqT, uh path: memsT0+WcT)
        # balanced across the two DMA queues, chunked so each matmul starts
        # as soon as its chunk lands
        bc2 = load(const, [128, 2], F32, bc_d.ap(), "bc2")
        xT0c, wqTc, memsT0c, wcTc = [None] * 4, [None] * 4, [None] * 2, [None] * 2
        wcTc[0] = load(const, [128, D], BF16, WcT_d.ap()[0], "wcTc0", nc.gpsimd)
        memsT0c[0] = load(pin, [128, S], BF16, memsT_d.ap()[0][0], "mT0c0",
                          nc.gpsimd)
        for ic in range(2):
            xT0c[ic] = load(pin, [128, TT], BF16, xT_d.ap()[0][ic], f"xT0c{ic}")
            wqTc[ic] = load(const, [128, D], BF16, WqT_d.ap()[ic], f"wqTc{ic}")
        for ic in range(2, 4):
            xT0c[ic] = load(pin, [128, TT], BF16, xT_d.ap()[0][ic], f"xT0c{ic}",
                            nc.gpsimd)
            wqTc[ic] = load(const, [128, D], BF16, WqT_d.ap()[ic], f"wqTc{ic}",
                            nc.gpsimd)
        memsT0c[1] = load(pin, [128, S], BF16, memsT_d.ap()[0][1], "mT0c1")
        wcTc[1] = load(const, [128, D], BF16, WcT_d.ap()[1], "wcTc1")
        vcols = load(const, [128, 2, 32, 32], BF16, vcols_d.ap(), "vcols")
        maskadds = [load(pin, [1, S], BF16, maskadd_d.ap()[j], "maskadd")
                    for j in range(NJ)]
        ones1 = load(const, [1, 128], BF16, ones_d.ap(), "ones1")
        boutw = load(const, [1, IN], BF16, boutw_d.ap(), "boutw")

        xT1c = [load(pin, [128, TT], BF16, xT_d.ap()[1][ic], f"xT1c{ic}",
                     nc.gpsimd) for ic in range(4)]
        memsT1c = [load(pin, [128, S], BF16, memsT_d.ap()[1][mc], f"mT1c{mc}",
                        nc.gpsimd) for mc in range(2)]
        woCT = load(const, [128, 2, IN], BF16, WoCT_d.ap(), "woCT", nc.gpsimd)
        woXT = load(const, [128, 4, IN], BF16, WoXT_d.ap(), "woXT", nc.gpsimd)
        ident = load(const, [128, 128], BF16, ident_d.ap(), "ident", nc.gpsimd)
        memsLs = [load(pin, [128, 4, D], BF16, memsL_d.ap()[j], "memsL",
                       nc.gpsimd) for j in range(NJ)]
        xTc = [xT0c, xT1c]
        memsTc = [memsT0c, memsT1c]

        def phase1(j):
            """wq [d_half, h, t] and uh [d_half, h, s] for tile j."""
            wq_ps = psWC.tile([128, 2 * TT], F32, tag="wqc_ps")
            for h in range(2):
                for ic in range(4):
                    nc.tensor.matmul(wq_ps[:, h * TT:(h + 1) * TT],
                                     wqTc[ic][:, h * 128:(h + 1) * 128],
                                     xTc[j][ic][...],
                                     start=(ic == 0), stop=(ic == 3))
            wq_sb = pmid.tile([128, 2, TT], F32, tag="wq_sb")
            nc.vector.tensor_copy(wq_sb[...], wq_ps[...])

            uh_sb = pmid.tile([128, 2, S], BF16, tag="uh_sb")
            for h in range(2):
                uh_ps = psU.tile([128, S], F32, tag="uh_ps")
                for mc in range(2):
                    nc.tensor.matmul(uh_ps[...],
                                     wcTc[mc][:, h * 128:(h + 1) * 128],
                                     memsTc[j][mc][...],
                                     start=(mc == 0), stop=(mc == 1))
                nc.vector.tensor_scalar_add(uh_sb[:, h, :], uh_ps[...],
                                            bc2[:, h:h + 1])
            return wq_sb, uh_sb

        p1 = [phase1(0), None]
        groups_of = [GROUPS_HEAD, GROUPS_TAIL]
        toff = [np.cumsum([0] + g).tolist() for g in groups_of]
        align_pss = [None, None]

        def emit_group(j, gi):
            wq_sb, uh_sb = p1[j]
            gsz = groups_of[j][gi]
            t0 = toff[j][gi]
            if align_pss[j] is None:
                align_pss[j] = psA.tile([128, S], F32, tag="align_ps",
                                        name=f"align_ps{j}")
            align_ps = align_pss[j]
            z = pz.tile([128, gsz * 2 * S], BF16, tag="z",
                        padded_shape=[128, GMAX * 2 * S])
            for jj in range(gsz):
                t = t0 + jj
                for h in range(2):
                    nc.vector.tensor_scalar_add(
                        z[:, (2 * jj + h) * S:(2 * jj + h + 1) * S],
                        uh_sb[:, h, :], wq_sb[:, h, t:t + 1])
            th = pth.tile([128, gsz * 2 * S], BF16, tag="th",
                          padded_shape=[128, GMAX * 2 * S])
            nc.scalar.activation(th[...], z[...], TANH)
            for jj in range(gsz):
                t = t0 + jj
                k, c = t // 32, t % 32
                for h in range(2):
                    nc.tensor.matmul(
                        align_ps[32 * k:32 * (k + 1), :],
                        vcols[:, h, c, :],
                        th[:, (2 * jj + h) * S:(2 * jj + h + 1) * S],
                        start=(c == 0 and h == 0),
                        stop=(c == 31 and h == 1),
                        tile_position=(0, 32 * k))

        def emit_epilogue(j):
            memsL = memsLs[j]
            align_ps = align_pss[j]

            # additive mask into PSUM: align += ones^T @ maskadd (0 / -30)
            nc.tensor.matmul(align_ps[...], ones1[...], maskadds[j][...],
                             start=False, stop=True, skip_group_check=True)

            # softmax (no max-sub: |align| <= ~10); sum fused into the exp
            av_e = pep.tile([128, S], F32, tag="av_e")
            ssum = pep.tile([128, 1], F32, tag="ssum")
            nc.scalar.activation(av_e[...], align_ps[...], EXP,
                                 accum_out=ssum[...])

            # x-part of the output projection does not depend on softmax;
            # one accumulation group for the whole bank (start=True clears
            # has_written bank-wide, so only the very first matmul starts)
            at_ps = psO.tile([128, 4 * TT], F32, tag="at_ps")
            for oc in range(4):
                for ic in range(4):
                    nc.tensor.matmul(at_ps[:, oc * TT:(oc + 1) * TT],
                                     woXT[:, ic, oc * 128:(oc + 1) * 128],
                                     xTc[j][ic][...],
                                     start=(oc == 0 and ic == 0), stop=False)

            rcp = pep.tile([128, 1], F32, tag="rcp")
            nc.vector.reciprocal(rcp[...], ssum[...])
            # bf16 copy first: it gates the transpose -> c chain
            av_bf = pep.tile([128, S], BF16, tag="av_bf")
            nc.vector.tensor_scalar_mul(av_bf[...], av_e[...], rcp[...])

            # transpose av -> [s, t]; one tile per s-block so the c matmul
            # can start as soon as its block is ready
            avTs = []
            for sb in range(4):
                tp = psT.tile([128, 128], BF16, tag="tp")
                nc.tensor.transpose(tp[...], av_bf[:, sb * 128:(sb + 1) * 128],
                                    ident[...])
                avT = pep.tile([128, TT], BF16, tag=f"avT{sb}")
                nc.vector.tensor_copy(avT[...], tp[...])
                avTs.append(avT)

            # align_vectors output (overlaps the c matmuls)
            av = pep.tile([128, S], F32, tag="av")
            nc.vector.tensor_scalar_mul(av[...], av_e[...], rcp[...])
            nc.sync.dma_start(align_d.ap()[j], av[...])

            # c[t,m] laid out [m_half(128), mh, t]
            c_ps = psWC.tile([128, 2 * TT], F32, tag="wqc_ps")
            for mh in range(2):
                for sb in range(4):
                    nc.tensor.matmul(c_ps[:, mh * TT:(mh + 1) * TT],
                                     memsL[:, sb, mh * 128:(mh + 1) * 128],
                                     avTs[sb][...],
                                     start=(sb == 0), stop=(sb == 3))
            c_bf = pep.tile([128, 2, TT], BF16, tag="c_bf")
            nc.vector.tensor_copy(c_bf[...], c_ps[...])

            # c-part + bout accumulate onto the x-part; per-oc chains so
            # eviction and output DMA pipeline with the remaining matmuls
            for oc in range(4):
                for mh in range(2):
                    nc.tensor.matmul(at_ps[:, oc * TT:(oc + 1) * TT],
                                     woCT[:, mh, oc * 128:(oc + 1) * 128],
                                     c_bf[:, mh, :],
                                     start=False, stop=False)
                nc.tensor.matmul(at_ps[:, oc * TT:(oc + 1) * TT],
                                 boutw[:, oc * 128:(oc + 1) * 128],
                                 ones1[...],
                                 start=False, stop=(oc == 3),
                                 skip_group_check=True)
                attn_sb = pep.tile([128, TT], F32, tag=f"attn_sb{oc}")
                nc.vector.tensor_copy(attn_sb[...],
                                      at_ps[:, oc * TT:(oc + 1) * TT])
                nc.sync.dma_start(attn_d.ap()[j][:, oc, :], attn_sb[...])

        # schedule: tile-0 groups (tile-1 prologue injected at group 6);
        # tile-1's first groups are emitted before tile-0's epilogue so the
        # DVE keeps feeding the ACT across the tile transition
        for gi in range(len(GROUPS_HEAD)):
            if gi == 6:
                p1[1] = phase1(1)
            emit_group(0, gi)
        emit_group(1, 0)
        emit_group(1, 1)
        emit_epilogue(0)
        for gi in range(2, len(GROUPS_TAIL)):
            emit_group(1, gi)
        emit_epilogue(1)

    nc.compile()
    return nc


def _to_chunks(a, nch):
    """[nch*128, F] -> [nch, 128, F] (partition-chunked SBUF layout)."""
    return np.ascontiguousarray(a.reshape(nch, 128, a.shape[-1]))


def _to_pcf(a, nch):
    """[nch*128, F] -> [128, nch, F] (single-tile chunked free layout)."""
    return np.ascontiguousarray(a.reshape(nch, 128, a.shape[-1]).transpose(1, 0, 2))


def _prep_inputs(inputs, mems, mem_masks, Wq, Wc, bc, v, Wout, bout):
    x = np.ascontiguousarray(np.asarray(inputs, dtype=np.float32))
    mems = np.ascontiguousarray(np.asarray(mems, dtype=np.float32))
    L = np.asarray(mem_masks).astype(np.int64)
    Wq = np.asarray(Wq, dtype=np.float32)
    Wc = np.asarray(Wc, dtype=np.float32)
    bc = np.asarray(bc, dtype=np.float32)
    v = np.asarray(v, dtype=np.float32)
    Wout = np.asarray(Wout, dtype=np.float32)
    bout = np.asarray(bout, dtype=np.float32)

    WqT = _to_chunks(np.ascontiguousarray(Wq.T), 4).astype(BF)      # [4,128,D]
    WcT = _to_chunks(np.ascontiguousarray(Wc.T), 2).astype(BF)      # [2,128,D]
    WoCT = _to_pcf(np.ascontiguousarray(Wout[:, :D].T), 2).astype(BF)
    WoXT = _to_pcf(np.ascontiguousarray(Wout[:, D:].T), 4).astype(BF)
    ident = np.eye(128, dtype=np.float32).astype(BF)
    bc2 = np.ascontiguousarray(bc.reshape(2, 128).T).astype(np.float32)
    vcols = np.zeros((128, 2, 32, 32), np.float32)
    for h in range(2):
        for c in range(32):
            vcols[:, h, c, c] = v[h * 128:(h + 1) * 128]
    vcols = vcols.astype(BF)

    shared = dict(WqT=WqT, WcT=WcT, vcols=vcols, WoCT=WoCT, WoXT=WoXT,
                  ident=ident, bc2=bc2,
                  boutw=bout.reshape(1, IN).astype(BF),
                  ones1=np.ones((1, 128), np.float32).astype(BF))

    tiles = [(b, tt) for b in range(B) for tt in range(T // TT)]
    in_maps = []
    for core in range(NC):
        xT = np.zeros((NJ, 4, 128, TT), np.float32)
        memsT = np.zeros((NJ, 2, 128, S), np.float32)
        memsL = np.zeros((NJ, 128, 4, D), np.float32)
        maskadd = np.zeros((NJ, 1, S), np.float32)
        for j in range(NJ):
            b, tt = tiles[core * NJ + j]
            xT[j] = _to_chunks(
                np.ascontiguousarray(x[b, tt * TT:(tt + 1) * TT, :].T), 4)
            memsT[j] = _to_chunks(np.ascontiguousarray(mems[b].T), 2)
            memsL[j] = _to_pcf(mems[b], 4)
            maskadd[j, 0, :] = np.where(np.arange(S) < int(L[b]), 0.0, -30.0)
        m = dict(shared)
        m["xT"] = np.ascontiguousarray(xT).astype(BF)
        m["memsT"] = np.ascontiguousarray(memsT).astype(BF)
        m["memsL"] = np.ascontiguousarray(memsL).astype(BF)
        m["maskadd"] = np.ascontiguousarray(maskadd).astype(BF)
        in_maps.append(m)
    return in_maps, tiles


def kernel(**inputs):
    global _BUILT, LAST_RESULT
    in_maps, tiles = _prep_inputs(**inputs)
    if _BUILT is None:
        _BUILT = _build()
    res = run_bass_kernel_spmd(_BUILT, in_maps, core_ids=list(range(NC)))
    LAST_RESULT = res

    attn_h = np.zeros((B, T, IN), np.float32)
    align_v = np.zeros((B, T, S), np.float32)
    for core in range(NC):
        for j in range(NJ):
            b, tt = tiles[core * NJ + j]
            at = res.results[core]["attn_outT"][j]        # [128(p), 4(oc), 128(t)]
            attn_h[b, tt * TT:(tt + 1) * TT, :] = \
                np.transpose(at, (2, 1, 0)).reshape(TT, IN)
            align_v[b, tt * TT:(tt + 1) * TT, :] = res.results[core]["align_out"][j]
    return attn_h, align_v


# revision 30
# speedup vs baseline: 1.0282x; 1.0038x over previous
"""Bahdanau additive attention on 8 TRN2 NeuronCores (Bass/Tile).

Reference computation (B=4, T=512, S=512, D=256, IN=512):
    wq[b,t,d]   = sum_i x[b,t,i]   * Wq[d,i]
    uh[b,s,d]   = sum_m mems[b,s,m]* Wc[d,m] + bc[d]
    align[b,t,s]= sum_d v[d] * tanh(wq[b,t,d] + uh[b,s,d])     (masked s>=L_b -> -inf)
    av          = softmax_s(align)
    c[b,t,m]    = sum_s av[b,t,s] * mems[b,s,m]
    attn[b,t,o] = sum_k [c|x][b,t,k] * Wout[o,k] + bout[o]
    returns (attn, av)

Sharding: 16 (batch, t-tile-of-128) blocks, 2 per core (pure data parallel,
no collectives).  Per (t, d-half): DVE broadcast-add z = uh + wq[t] (bf16,
4x mode), ACT tanh batched over many slices (one big ACTIVATE), PE reduces
over the d-partition dim with a 32-column one-hot v weight into the PSUM
row for t.  Mask is a rank-1 additive matmul on the align PSUM (0 / -30);
softmax sum is fused into the Exp via accum_out; bout is a rank-1 matmul.
All matmul inputs bf16, accumulation fp32.  DRAM inputs are pre-laid
host-side in the exact SBUF [128, ...] layout so every DMA is contiguous.
"""
import numpy as np
import ml_dtypes
from contextlib import ExitStack

import concourse.bass as bass
import concourse.bacc as bacc
import concourse.mybir as mybir
import concourse.tile as tile
from concourse.bass_utils import run_bass_kernel_spmd

F32 = mybir.dt.float32
BF16 = mybir.dt.bfloat16
TANH = mybir.ActivationFunctionType.Tanh
EXP = mybir.ActivationFunctionType.Exp
BF = ml_dtypes.bfloat16

B, T, S, D, IN = 4, 512, 512, 256, 512
NC = 8           # cores
NJ = 2           # t-tiles per core
TT = 128         # t rows per tile

# ACT batch sizes (t's per ACTIVATE).  Small groups at the edges: the first
# tanh starts sooner (less z to build) and after the last tanh the PE's
# v-matmul chase is short.
GROUPS_HEAD = [2, 2, 4] + [8] * 15
GROUPS_TAIL = [8] * 15 + [4, 2, 2]
assert sum(GROUPS_HEAD) == TT and sum(GROUPS_TAIL) == TT
GMAX = max(GROUPS_HEAD)

_BUILT = None
LAST_RESULT = None


def _build():
    nc = bacc.Bacc("TRN2", target_bir_lowering=False, debug=False,
                   enable_asserts=False, num_devices=NC)

    # all inputs pre-laid in SBUF layout [128 partitions, free...]
    xT_d = nc.dram_tensor("xT", [NJ, 4, 128, TT], BF16, kind="ExternalInput")
    memsT_d = nc.dram_tensor("memsT", [NJ, 2, 128, S], BF16, kind="ExternalInput")
    memsL_d = nc.dram_tensor("memsL", [NJ, 128, 4, D], BF16, kind="ExternalInput")
    maskadd_d = nc.dram_tensor("maskadd", [NJ, 1, S], BF16, kind="ExternalInput")
    ones_d = nc.dram_tensor("ones1", [1, 128], BF16, kind="ExternalInput")
    boutw_d = nc.dram_tensor("boutw", [1, IN], BF16, kind="ExternalInput")
    WqT_d = nc.dram_tensor("WqT", [4, 128, D], BF16, kind="ExternalInput")
    WcT_d = nc.dram_tensor("WcT", [2, 128, D], BF16, kind="ExternalInput")
    vcols_d = nc.dram_tensor("vcols", [128, 2, 32, 32], BF16, kind="ExternalInput")
    WoCT_d = nc.dram_tensor("WoCT", [128, 2, IN], BF16, kind="ExternalInput")
    WoXT_d = nc.dram_tensor("WoXT", [128, 4, IN], BF16, kind="ExternalInput")
    ident_d = nc.dram_tensor("ident", [128, 128], BF16, kind="ExternalInput")
    bc_d = nc.dram_tensor("bc2", [128, 2], F32, kind="ExternalInput")

    attn_d = nc.dram_tensor("attn_outT", [NJ, 128, 4, TT], F32, kind="ExternalOutput")
    align_d = nc.dram_tensor("align_out", [NJ, 128, S], F32, kind="ExternalOutput")

    with tile.TileContext(nc) as tc, ExitStack() as ctx:
        const = ctx.enter_context(tc.tile_pool(name="const", bufs=1))
        pin = ctx.enter_context(tc.tile_pool(name="pin", bufs=NJ))
        pmid = ctx.enter_context(tc.tile_pool(name="pmid", bufs=NJ))
        pz = ctx.enter_context(tc.tile_pool(name="pz", bufs=4))
        pth = ctx.enter_context(tc.tile_pool(name="pth", bufs=4))
        pep = ctx.enter_context(tc.tile_pool(name="pep", bufs=NJ))
        psA = ctx.enter_context(tc.tile_pool(name="psA", bufs=NJ, space="PSUM"))
        psU = ctx.enter_context(tc.tile_pool(name="psU", bufs=2, space="PSUM"))
        psWC = ctx.enter_context(tc.tile_pool(name="psWC", bufs=1, space="PSUM"))
        psT = ctx.enter_context(tc.tile_pool(name="psT", bufs=2, space="PSUM"))
        psO = ctx.enter_context(tc.tile_pool(name="psO", bufs=1, space="PSUM"))

        def load(pool, shape, dt, src, tag, engine=None):
            t = pool.tile(shape, dt, tag=tag)
            (engine or nc.sync).dma_start(t[...], src)
            return t

        # startup-critical loads (wq path: xT0+WqT, uh path: memsT0+WcT)
        # balanced across the two DMA queues, chunked so each matmul starts
        # as soon as its chunk lands
        bc2 = load(const, [128, 2], F32, bc_d.ap(), "bc2")
        xT0c, wqTc, memsT0c, wcTc = [None] * 4, [None] * 4, [None] * 2, [None] * 2
        wcTc[0] = load(const, [128, D], BF16, WcT_d.ap()[0], "wcTc0", nc.gpsimd)
        memsT0c[0] = load(pin, [128, S], BF16, memsT_d.ap()[0][0], "mT0c0",
                          nc.gpsimd)
        xT0c[0] = load(pin, [128, TT], BF16, xT_d.ap()[0][0], "xT0c0")
        wqTc[0] = load(const, [128, D], BF16, WqT_d.ap()[0], "wqTc0")
        memsT0c[1] = load(pin, [128, S], BF16, memsT_d.ap()[0][1], "mT0c1")
        wcTc[1] = load(const, [128, D], BF16, WcT_d.ap()[1], "wcTc1")
        xT0c[1] = load(pin, [128, TT], BF16, xT_d.ap()[0][1], "xT0c1")
        wqTc[1] = load(const, [128, D], BF16, WqT_d.ap()[1], "wqTc1")
        for ic in range(2, 4):
            xT0c[ic] = load(pin, [128, TT], BF16, xT_d.ap()[0][ic], f"xT0c{ic}",
                            nc.gpsimd)
            wqTc[ic] = load(const, [128, D], BF16, WqT_d.ap()[ic], f"wqTc{ic}",
                            nc.gpsimd)
        vcols = load(const, [128, 2, 32, 32], BF16, vcols_d.ap(), "vcols")
        maskadds = [load(pin, [1, S], BF16, maskadd_d.ap()[j], "maskadd")
                    for j in range(NJ)]
        ones1 = load(const, [1, 128], BF16, ones_d.ap(), "ones1")
        boutw = load(const, [1, IN], BF16, boutw_d.ap(), "boutw")

        xT1c = [load(pin, [128, TT], BF16, xT_d.ap()[1][ic], f"xT1c{ic}",
                     nc.gpsimd) for ic in range(4)]
        memsT1c = [load(pin, [128, S], BF16, memsT_d.ap()[1][mc], f"mT1c{mc}",
                        nc.gpsimd) for mc in range(2)]
        woCT = load(const, [128, 2, IN], BF16, WoCT_d.ap(), "woCT", nc.gpsimd)
        woXT = load(const, [128, 4, IN], BF16, WoXT_d.ap(), "woXT", nc.gpsimd)
        ident = load(const, [128, 128], BF16, ident_d.ap(), "ident", nc.gpsimd)
        memsLs = [load(pin, [128, 4, D], BF16, memsL_d.ap()[j], "memsL",
                       nc.gpsimd) for j in range(NJ)]
        xTc = [xT0c, xT1c]
        memsTc = [memsT0c, memsT1c]

        def phase1(j):
            """wq [d_half, h, t] and uh [d_half, h, s] for tile j."""
            wq_ps = psWC.tile([128, 2 * TT], F32, tag="wqc_ps")
            for h in range(2):
                for ic in range(4):
                    nc.tensor.matmul(wq_ps[:, h * TT:(h + 1) * TT],
                                     wqTc[ic][:, h * 128:(h + 1) * 128],
                                     xTc[j][ic][...],
                                     start=(ic == 0), stop=(ic == 3))
            wq_sb = pmid.tile([128, 2, TT], F32, tag="wq_sb")
            nc.vector.tensor_copy(wq_sb[...], wq_ps[...])

            uh_sb = pmid.tile([128, 2, S], BF16, tag="uh_sb")
            for h in range(2):
                uh_ps = psU.tile([128, S], F32, tag="uh_ps")
                for mc in range(2):
                    nc.tensor.matmul(uh_ps[...],
                                     wcTc[mc][:, h * 128:(h + 1) * 128],
                                     memsTc[j][mc][...],
                                     start=(mc == 0), stop=(mc == 1))
                nc.vector.tensor_scalar_add(uh_sb[:, h, :], uh_ps[...],
                                            bc2[:, h:h + 1])
            return wq_sb, uh_sb

        p1 = [phase1(0), None]
        groups_of = [GROUPS_HEAD, GROUPS_TAIL]
        toff = [np.cumsum([0] + g).tolist() for g in groups_of]
        align_pss = [None, None]

        def emit_group(j, gi):
            wq_sb, uh_sb = p1[j]
            gsz = groups_of[j][gi]
            t0 = toff[j][gi]
            if align_pss[j] is None:
                align_pss[j] = psA.tile([128, S], F32, tag="align_ps",
                                        name=f"align_ps{j}")
            align_ps = align_pss[j]
            z = pz.tile([128, gsz * 2 * S], BF16, tag="z",
                        padded_shape=[128, GMAX * 2 * S])
            for jj in range(gsz):
                t = t0 + jj
                for h in range(2):
                    nc.vector.tensor_scalar_add(
                        z[:, (2 * jj + h) * S:(2 * jj + h + 1) * S],
                        uh_sb[:, h, :], wq_sb[:, h, t:t + 1])
            th = pth.tile([128, gsz * 2 * S], BF16, tag="th",
                          padded_shape=[128, GMAX * 2 * S])
            nc.scalar.activation(th[...], z[...], TANH)
            for jj in range(gsz):
                t = t0 + jj
                k, c = t // 32, t % 32
                for h in range(2):
                    nc.tensor.matmul(
                        align_ps[32 * k:32 * (k + 1), :],
                        vcols[:, h, c, :],
                        th[:, (2 * jj + h) * S:(2 * jj + h + 1) * S],
                        start=(c == 0 and h == 0),
                        stop=(c == 31 and h == 1),
                        tile_position=(0, 32 * k))

        def emit_epilogue(j):
            memsL = memsLs[j]
            align_ps = align_pss[j]

            # additive mask into PSUM: align += ones^T @ maskadd (0 / -30)
            nc.tensor.matmul(align_ps[...], ones1[...], maskadds[j][...],
                             start=False, stop=True, skip_group_check=True)

            # softmax (no max-sub: |align| <= ~10); sum fused into the exp
            av_e = pep.tile([128, S], F32, tag="av_e")
            ssum = pep.tile([128, 1], F32, tag="ssum")
            nc.scalar.activation(av_e[...], align_ps[...], EXP,
                                 accum_out=ssum[...])

            # x-part of the output projection does not depend on softmax;
            # one accumulation group for the whole bank (start=True clears
            # has_written bank-wide, so only the very first matmul starts)
            at_ps = psO.tile([128, 4 * TT], F32, tag="at_ps")
            for oc in range(4):
                for ic in range(4):
                    nc.tensor.matmul(at_ps[:, oc * TT:(oc + 1) * TT],
                                     woXT[:, ic, oc * 128:(oc + 1) * 128],
                                     xTc[j][ic][...],
                                     start=(oc == 0 and ic == 0), stop=False)

            rcp = pep.tile([128, 1], F32, tag="rcp")
            nc.vector.reciprocal(rcp[...], ssum[...])
            # bf16 copy first: it gates the transpose -> c chain
            av_bf = pep.tile([128, S], BF16, tag="av_bf")
            nc.vector.tensor_scalar_mul(av_bf[...], av_e[...], rcp[...])

            # transpose av -> [s, t]; one tile per s-block so the c matmul
            # can start as soon as its block is ready
            avTs = []
            for sb in range(4):
                tp = psT.tile([128, 128], BF16, tag="tp")
                nc.tensor.transpose(tp[...], av_bf[:, sb * 128:(sb + 1) * 128],
                                    ident[...])
                avT = pep.tile([128, TT], BF16, tag=f"avT{sb}")
                nc.vector.tensor_copy(avT[...], tp[...])
                avTs.append(avT)

            # align_vectors output (overlaps the c matmuls)
            av = pep.tile([128, S], F32, tag="av")
            nc.vector.tensor_scalar_mul(av[...], av_e[...], rcp[...])
            nc.sync.dma_start(align_d.ap()[j], av[...])

            # c[t,m] laid out [m_half(128), mh, t]
            c_ps = psWC.tile([128, 2 * TT], F32, tag="wqc_ps")
            for mh in range(2):
                for sb in range(4):
                    nc.tensor.matmul(c_ps[:, mh * TT:(mh + 1) * TT],
                                     memsL[:, sb, mh * 128:(mh + 1) * 128],
                                     avTs[sb][...],
                                     start=(sb == 0), stop=(sb == 3))
            c_bf = pep.tile([128, 2, TT], BF16, tag="c_bf")
            nc.vector.tensor_copy(c_bf[...], c_ps[...])

            # c-part + bout accumulate onto the x-part; per-oc chains so
            # eviction and output DMA pipeline with the remaining matmuls
            for oc in range(4):
                for mh in range(2):
                    nc.tensor.matmul(at_ps[:, oc * TT:(oc + 1) * TT],
                                     woCT[:, mh, oc * 128:(oc + 1) * 128],
                                     c_bf[:, mh, :],
                                     start=False, stop=False)
                nc.tensor.matmul(at_ps[:, oc * TT:(oc + 1) * TT],
                                 boutw[:, oc * 128:(oc + 1) * 128],
                                 ones1[...],
                                 start=False, stop=(oc == 3),
                                 skip_group_check=True)
                attn_sb = pep.tile([128, TT], F32, tag=f"attn_sb{oc}")
                nc.vector.tensor_copy(attn_sb[...],
                                      at_ps[:, oc * TT:(oc + 1) * TT])
                nc.sync.dma_start(attn_d.ap()[j][:, oc, :], attn_sb[...])

        # schedule: tile-0 groups (tile-1 prologue injected at group 6);
        # tile-1's first groups are emitted before tile-0's epilogue so the
        # DVE keeps feeding the ACT across the tile transition
        for gi in range(len(GROUPS_HEAD)):
            if gi == 6:
                p1[1] = phase1(1)
            emit_group(0, gi)
        emit_group(1, 0)
        emit_group(1, 1)
        emit_epilogue(0)
        for gi in range(2, len(GROUPS_TAIL)):
            emit_group(1, gi)
        emit_epilogue(1)

    nc.compile()
    return nc


def _to_chunks(a, nch):
    """[nch*128, F] -> [nch, 128, F] (partition-chunked SBUF layout)."""
    return np.ascontiguousarray(a.reshape(nch, 128, a.shape[-1]))


def _to_pcf(a, nch):
    """[nch*128, F] -> [128, nch, F] (single-tile chunked free layout)."""
    return np.ascontiguousarray(a.reshape(nch, 128, a.shape[-1]).transpose(1, 0, 2))


def _prep_inputs(inputs, mems, mem_masks, Wq, Wc, bc, v, Wout, bout):
    x = np.ascontiguousarray(np.asarray(inputs, dtype=np.float32))
    mems = np.ascontiguousarray(np.asarray(mems, dtype=np.float32))
    L = np.asarray(mem_masks).astype(np.int64)
    Wq = np.asarray(Wq, dtype=np.float32)
    Wc = np.asarray(Wc, dtype=np.float32)
    bc = np.asarray(bc, dtype=np.float32)
    v = np.asarray(v, dtype=np.float32)
    Wout = np.asarray(Wout, dtype=np.float32)
    bout = np.asarray(bout, dtype=np.float32)

    WqT = _to_chunks(np.ascontiguousarray(Wq.T), 4).astype(BF)      # [4,128,D]
    WcT = _to_chunks(np.ascontiguousarray(Wc.T), 2).astype(BF)      # [2,128,D]
    WoCT = _to_pcf(np.ascontiguousarray(Wout[:, :D].T), 2).astype(BF)
    WoXT = _to_pcf(np.ascontiguousarray(Wout[:, D:].T), 4).astype(BF)
    ident = np.eye(128, dtype=np.float32).astype(BF)
    bc2 = np.ascontiguousarray(bc.reshape(2, 128).T).astype(np.float32)
    vcols = np.zeros((128, 2, 32, 32), np.float32)
    for h in range(2):
        for c in range(32):
            vcols[:, h, c, c] = v[h * 128:(h + 1) * 128]
    vcols = vcols.astype(BF)

    shared = dict(WqT=WqT, WcT=WcT, vcols=vcols, WoCT=WoCT, WoXT=WoXT,
                  ident=ident, bc2=bc2,
                  boutw=bout.reshape(1, IN).astype(BF),
                  ones1=np.ones((1, 128), np.float32).astype(BF))

    tiles = [(b, tt) for b in range(B) for tt in range(T // TT)]
    in_maps = []
    for core in range(NC):
        xT = np.zeros((NJ, 4, 128, TT), np.float32)
        memsT = np.zeros((NJ, 2, 128, S), np.float32)
        memsL = np.zeros((NJ, 128, 4, D), np.float32)
        maskadd = np.zeros((NJ, 1, S), np.float32)
        for j in range(NJ):
            b, tt = tiles[core * NJ + j]
            xT[j] = _to_chunks(
                np.ascontiguousarray(x[b, tt * TT:(tt + 1) * TT, :].T), 4)
            memsT[j] = _to_chunks(np.ascontiguousarray(mems[b].T), 2)
            memsL[j] = _to_pcf(mems[b], 4)
            maskadd[j, 0, :] = np.where(np.arange(S) < int(L[b]), 0.0, -30.0)
        m = dict(shared)
        m["xT"] = np.ascontiguousarray(xT).astype(BF)
        m["memsT"] = np.ascontiguousarray(memsT).astype(BF)
        m["memsL"] = np.ascontiguousarray(memsL).astype(BF)
        m["maskadd"] = np.ascontiguousarray(maskadd).astype(BF)
        in_maps.append(m)
    return in_maps, tiles


def kernel(**inputs):
    global _BUILT, LAST_RESULT
    in_maps, tiles = _prep_inputs(**inputs)
    if _BUILT is None:
        _BUILT = _build()
    res = run_bass_kernel_spmd(_BUILT, in_maps, core_ids=list(range(NC)))
    LAST_RESULT = res

    attn_h = np.zeros((B, T, IN), np.float32)
    align_v = np.zeros((B, T, S), np.float32)
    for core in range(NC):
        for j in range(NJ):
            b, tt = tiles[core * NJ + j]
            at = res.results[core]["attn_outT"][j]        # [128(p), 4(oc), 128(t)]
            attn_h[b, tt * TT:(tt + 1) * TT, :] = \
                np.transpose(at, (2, 1, 0)).reshape(TT, IN)
            align_v[b, tt * TT:(tt + 1) * TT, :] = res.results[core]["align_out"][j]
    return attn_h, align_v
